# revision 5
# baseline (speedup 1.0000x reference)
"""Bucket-indexed spatially-varying (channel-shared) 5x5 convolution on 8 trn2 cores.

out[b,c,y,x] = sum_{i,j} pad(input)[b,c,y+i,x+j] * kernel_bank[buckets[b,y,x], i, j]

Strategy (data-parallel over batch, one image per core):
  * Layout: partition dim = image row y (two 128-row tiles), free dim = (channel, x).
  * Per-pixel kernels ("Wmap") built on device: buckets -> one-hot (DVE is_equal)
    -> PE matmul against the [64,25] bank -> [25, Npix] tap-major weight map,
    staged to DRAM as [y, tap, x].
  * Conv: for each tap and each x column, one fused scalar_tensor_tensor op:
      acc[y, :, x] = (x_shift[y, :, x+dx] * w[y]) + acc[y, :, x]
    where w is a per-partition (per-row) scalar AP - exact fp32 FMA in one
    DVE pass. dy shifts are handled by loading 5 row-shifted copies of the
    input tile (partition shifts are impossible inside SBUF ops).
"""

import sys

sys.path.insert(0, "/opt/trn_rl_repo")

import numpy as np

B, C, H, W = 8, 128, 256, 256
K, NB = 5, 64
PAD = (K - 1) // 2  # 2
HP, WP = H + 2 * PAD, W + 2 * PAD  # 260, 260
N_CORES = 8
NT = K * K  # 25 taps

YT = 2  # y tiles of 128 rows
XW = 16  # x block width
NXB = W // XW  # 16

_CACHE = {}


def _build_nc():
    import concourse.bacc as bacc
    import concourse.mybir as mybir
    from concourse import tile

    f32 = mybir.dt.float32
    Alu = mybir.AluOpType

    nc = bacc.Bacc(None)

    xp = nc.dram_tensor("xp", [C, HP, WP], f32, kind="ExternalInput")
    bkf = nc.dram_tensor("bkf", [H, W], f32, kind="ExternalInput")
    bank = nc.dram_tensor("bank", [NB, NT], f32, kind="ExternalInput")
    iota = nc.dram_tensor("iota", [NB, 1], f32, kind="ExternalInput")
    y_out = nc.dram_tensor("y", [C, H, W], f32, kind="ExternalOutput")

    with tile.TileContext(nc) as tc:
        with tc.tile_pool(name="dram", bufs=1, space="DRAM") as dpool:
            # weight map staged in DRAM as [y, tap, x]
            wm_dram = dpool.tile([H, NT, W], f32)

            # ---------------- Phase A: build Wmap ----------------
            with (
                tc.tile_pool(name="wconst", bufs=1) as cpool,
                tc.tile_pool(name="wbuild", bufs=3) as wpool,
                tc.tile_pool(name="wpsum", bufs=4, space="PSUM") as pspool,
            ):
                bank_sb = cpool.tile([NB, NT], f32)
                nc.sync.dma_start(out=bank_sb[:], in_=bank[:])
                iota_sb = cpool.tile([NB, 1], f32)
                nc.sync.dma_start(out=iota_sb[:], in_=iota[:])

                GROWS = 16  # bucket rows per group
                GPIX = GROWS * W  # 4096
                for g in range(H // GROWS):  # 16 groups
                    brep = wpool.tile([NB, GPIX], f32, tag="brep")
                    nc.sync.dma_start(
                        out=brep[:],
                        in_=bkf[g * GROWS : (g + 1) * GROWS, :]
                        .rearrange("(o h) w -> o (h w)", o=1)
                        .broadcast_to((NB, GPIX)),
                    )
                    oh = wpool.tile([NB, GPIX], f32, tag="oh")
                    nc.vector.tensor_scalar(
                        out=oh[:],
                        in0=brep[:],
                        scalar1=iota_sb[:],
                        scalar2=None,
                        op0=Alu.is_equal,
                    )
                    for c8 in range(GPIX // 512):  # 8 chunks of 512 px (2 rows)
                        ps = pspool.tile([NT, 512], f32, tag="ps")
                        nc.tensor.matmul(
                            ps[:],
                            bank_sb[:],
                            oh[:, c8 * 512 : (c8 + 1) * 512],
                            start=True,
                            stop=True,
                        )
                        wms = wpool.tile([NT, 512], f32, tag="wms")
                        nc.scalar.copy(out=wms[:], in_=ps[:])
                        y0 = g * GROWS + c8 * 2
                        # keep the SBUF partition dim (t) first on both sides;
                        # an SBUF-side rearrange that moves the partition dim
                        # scrambles the transfer.
                        nc.sync.dma_start(
                            out=wm_dram[y0 : y0 + 2, :, :].rearrange(
                                "y t x -> t y x"
                            ),
                            in_=wms.rearrange("t (y x) -> t y x", y=2),
                        )

            # ---------------- Phase B: convolution ----------------
            with (
                tc.tile_pool(name="xs", bufs=2) as xpool,
                tc.tile_pool(name="wm", bufs=2) as wmpool,
                tc.tile_pool(name="acc", bufs=2) as apool,
            ):
                for yt in range(YT):
                    for xb in range(NXB):
                        x0 = xb * XW
                        wm_t = wmpool.tile([128, NT, XW], f32, tag="wm")
                        nc.sync.dma_start(
                            out=wm_t[:],
                            in_=wm_dram[yt * 128 : (yt + 1) * 128, :, x0 : x0 + XW],
                        )
                        xs = xpool.tile([128, K, C, XW + 2 * PAD], f32, tag="xs")
                        for dy in range(K):
                            nc.sync.dma_start(
                                out=xs[:, dy, :, :],
                                in_=xp[
                                    :,
                                    yt * 128 + dy : yt * 128 + dy + 128,
                                    x0 : x0 + XW + 2 * PAD,
                                ].rearrange("c y x -> y c x"),
                            )
                        acc = apool.tile([128, C, XW], f32, tag="acc")
                        for t in range(NT):
                            dy, dx = t // K, t % K
                            for ix in range(XW):
                                in0 = xs[:, dy, :, ix + dx]
                                w = wm_t[:, t, ix : ix + 1]
                                if t == 0:
                                    nc.vector.tensor_scalar(
                                        out=acc[:, :, ix],
                                        in0=in0,
                                        scalar1=w,
                                        scalar2=None,
                                        op0=Alu.mult,
                                    )
                                else:
                                    nc.vector.scalar_tensor_tensor(
                                        out=acc[:, :, ix],
                                        in0=in0,
                                        scalar=w,
                                        in1=acc[:, :, ix],
                                        op0=Alu.mult,
                                        op1=Alu.add,
                                    )
                        nc.sync.dma_start(
                            out=y_out[
                                :, yt * 128 : (yt + 1) * 128, x0 : x0 + XW
                            ].rearrange("c y x -> y c x"),
                            in_=acc[:],
                        )

    nc.finalize()
    return nc


def _get_nc():
    if "nc" not in _CACHE:
        _CACHE["nc"] = _build_nc()
    return _CACHE["nc"]


def kernel(input, kernel_bank, buckets):
    from concourse.bass_utils import run_bass_kernel_spmd

    nc = _get_nc()

    input = np.ascontiguousarray(input, dtype=np.float32)
    xpad = np.pad(input, ((0, 0), (0, 0), (PAD, PAD), (PAD, PAD)))
    bkf = np.ascontiguousarray(buckets, dtype=np.int32).astype(np.float32)
    bank2 = np.ascontiguousarray(kernel_bank, dtype=np.float32).reshape(NB, NT)
    iota64 = np.arange(NB, dtype=np.float32).reshape(NB, 1)

    in_maps = [
        {"xp": xpad[i], "bkf": bkf[i], "bank": bank2, "iota": iota64}
        for i in range(N_CORES)
    ]
    res = run_bass_kernel_spmd(nc, in_maps, list(range(N_CORES)))
    out = np.stack([res.results[i]["y"] for i in range(N_CORES)], axis=0)
    return out.astype(np.float32)


# revision 7
# speedup vs baseline: 2107.2859x; 2107.2859x over previous
"""Bucket-indexed spatially-varying (channel-shared) 5x5 convolution on 8 trn2 cores.

out[b,c,y,x] = sum_{i,j} pad(input)[b,c,y+i,x+j] * kernel_bank[buckets[b,y,x], i, j]

Strategy (data-parallel over batch, one image per core):
  * Layout: partition dim = image row y (two 128-row tiles), free dim = (channel, x).
  * Per-pixel kernels ("Wmap") built on device: buckets -> one-hot (DVE is_equal)
    -> PE matmul against the [64,25] bank -> [25, Npix] tap-major weight map,
    staged to DRAM as [y, tap, x].
  * Conv: for each tap and each x column, one fused scalar_tensor_tensor op:
      acc[y, :, x] = (x_shift[y, :, x+dx] * w[y]) + acc[y, :, x]
    where w is a per-partition (per-row) scalar AP - exact fp32 FMA in one
    DVE pass. dy shifts are handled by loading 5 row-shifted copies of the
    input tile (partition shifts are impossible inside SBUF ops).
"""

import sys

sys.path.insert(0, "/opt/trn_rl_repo")

import numpy as np

B, C, H, W = 8, 128, 256, 256
K, NB = 5, 64
PAD = (K - 1) // 2  # 2
HP, WP = H + 2 * PAD, W + 2 * PAD  # 260, 260
N_CORES = 8
NT = K * K  # 25 taps

YT = 2  # y tiles of 128 rows
XW = 16  # x block width
NXB = W // XW  # 16

_CACHE = {}


def _build_nc(conv_reps=1):
    import concourse.bacc as bacc
    import concourse.mybir as mybir
    from concourse import tile

    f32 = mybir.dt.float32
    Alu = mybir.AluOpType

    nc = bacc.Bacc(None)

    xp = nc.dram_tensor("xp", [C, HP, WP], f32, kind="ExternalInput")
    bkf = nc.dram_tensor("bkf", [H, W], f32, kind="ExternalInput")
    bank = nc.dram_tensor("bank", [NB, NT], f32, kind="ExternalInput")
    iota = nc.dram_tensor("iota", [NB, 1], f32, kind="ExternalInput")
    y_out = nc.dram_tensor("y", [C, H, W], f32, kind="ExternalOutput")

    with tile.TileContext(nc) as tc:
        with tc.tile_pool(name="dram", bufs=1, space="DRAM") as dpool:
            # weight map staged in DRAM as [y, tap, x]
            wm_dram = dpool.tile([H, NT, W], f32)

            # ---------------- Phase A: build Wmap ----------------
            with (
                tc.tile_pool(name="wconst", bufs=1) as cpool,
                tc.tile_pool(name="wbuild", bufs=3) as wpool,
                tc.tile_pool(name="wpsum", bufs=4, space="PSUM") as pspool,
            ):
                bank_sb = cpool.tile([NB, NT], f32)
                nc.sync.dma_start(out=bank_sb[:], in_=bank[:])
                iota_sb = cpool.tile([NB, 1], f32)
                nc.sync.dma_start(out=iota_sb[:], in_=iota[:])

                GROWS = 16  # bucket rows per group
                GPIX = GROWS * W  # 4096
                for g in range(H // GROWS):  # 16 groups
                    brep = wpool.tile([NB, GPIX], f32, tag="brep")
                    nc.sync.dma_start(
                        out=brep[:],
                        in_=bkf[g * GROWS : (g + 1) * GROWS, :]
                        .rearrange("(o h) w -> o (h w)", o=1)
                        .broadcast_to((NB, GPIX)),
                    )
                    oh = wpool.tile([NB, GPIX], f32, tag="oh")
                    nc.vector.tensor_scalar(
                        out=oh[:],
                        in0=brep[:],
                        scalar1=iota_sb[:],
                        scalar2=None,
                        op0=Alu.is_equal,
                    )
                    for c8 in range(GPIX // 512):  # 8 chunks of 512 px (2 rows)
                        ps = pspool.tile([NT, 512], f32, tag="ps")
                        nc.tensor.matmul(
                            ps[:],
                            bank_sb[:],
                            oh[:, c8 * 512 : (c8 + 1) * 512],
                            start=True,
                            stop=True,
                        )
                        wms = wpool.tile([NT, 512], f32, tag="wms")
                        nc.scalar.copy(out=wms[:], in_=ps[:])
                        y0 = g * GROWS + c8 * 2
                        # keep the SBUF partition dim (t) first on both sides;
                        # an SBUF-side rearrange that moves the partition dim
                        # scrambles the transfer.
                        nc.sync.dma_start(
                            out=wm_dram[y0 : y0 + 2, :, :].rearrange(
                                "y t x -> t y x"
                            ),
                            in_=wms.rearrange("t (y x) -> t y x", y=2),
                        )

            # ---------------- Phase B: convolution ----------------
            with (
                tc.tile_pool(name="xs", bufs=2) as xpool,
                tc.tile_pool(name="wm", bufs=2) as wmpool,
                tc.tile_pool(name="acc", bufs=2) as apool,
            ):
                for rep in range(conv_reps):
                  for yt in range(YT):
                    for xb in range(NXB):
                        x0 = xb * XW
                        wm_t = wmpool.tile([128, NT, XW], f32, tag="wm")
                        nc.sync.dma_start(
                            out=wm_t[:],
                            in_=wm_dram[yt * 128 : (yt + 1) * 128, :, x0 : x0 + XW],
                        )
                        xs = xpool.tile([128, K, C, XW + 2 * PAD], f32, tag="xs")
                        for dy in range(K):
                            nc.sync.dma_start(
                                out=xs[:, dy, :, :],
                                in_=xp[
                                    :,
                                    yt * 128 + dy : yt * 128 + dy + 128,
                                    x0 : x0 + XW + 2 * PAD,
                                ].rearrange("c y x -> y c x"),
                            )
                        acc = apool.tile([128, C, XW], f32, tag="acc")
                        for t in range(NT):
                            dy, dx = t // K, t % K
                            for ix in range(XW):
                                in0 = xs[:, dy, :, ix + dx]
                                w = wm_t[:, t, ix : ix + 1]
                                if t == 0:
                                    nc.vector.tensor_scalar(
                                        out=acc[:, :, ix],
                                        in0=in0,
                                        scalar1=w,
                                        scalar2=None,
                                        op0=Alu.mult,
                                    )
                                else:
                                    nc.vector.scalar_tensor_tensor(
                                        out=acc[:, :, ix],
                                        in0=in0,
                                        scalar=w,
                                        in1=acc[:, :, ix],
                                        op0=Alu.mult,
                                        op1=Alu.add,
                                    )
                        nc.sync.dma_start(
                            out=y_out[
                                :, yt * 128 : (yt + 1) * 128, x0 : x0 + XW
                            ].rearrange("c y x -> y c x"),
                            in_=acc[:],
                        )

    nc.finalize()
    return nc


def _get_nc():
    if "nc" not in _CACHE:
        _CACHE["nc"] = _build_nc()
    return _CACHE["nc"]


def kernel(input, kernel_bank, buckets):
    from concourse.bass_utils import run_bass_kernel_spmd

    nc = _get_nc()

    input = np.ascontiguousarray(input, dtype=np.float32)
    xpad = np.pad(input, ((0, 0), (0, 0), (PAD, PAD), (PAD, PAD)))
    bkf = np.ascontiguousarray(buckets, dtype=np.int32).astype(np.float32)
    bank2 = np.ascontiguousarray(kernel_bank, dtype=np.float32).reshape(NB, NT)
    iota64 = np.arange(NB, dtype=np.float32).reshape(NB, 1)

    in_maps = [
        {"xp": xpad[i], "bkf": bkf[i], "bank": bank2, "iota": iota64}
        for i in range(N_CORES)
    ]
    res = run_bass_kernel_spmd(nc, in_maps, list(range(N_CORES)))
    out = np.stack([res.results[i]["y"] for i in range(N_CORES)], axis=0)
    return out.astype(np.float32)
